# revision 26
# baseline (speedup 1.0000x reference)
"""NVFP4-style activation quantizer on 8 TRN2 NeuronCores (raw bass).

Reference semantics (per 16-element block, fp32):
    s_t  = max|x| / (6*448)                      (global, needs all-reduce)
    m_b  = max|x| over block
    inv  = 6 / (m_b / s_t)
    s_b  = fp8_e4m3_roundtrip(inv), guarded to 1.0 if 0/inf/nan
    out  = sign(x) * fp4_121(|x|/s_t * s_b) / s_b * s_t

Device algorithm (signed, select-free), per element:
    y  = x * c            with c = s_b / s_t  (per block)
    pa = bits(y) & 0x7f800000                  (= bits of 2^e of |y|)
    B  = max_int(pa + 0x0B400000, 0x4AC00000)  (= 3*2^21 * max(2^e, 1))
    t  = y + B            (fp32 RNE add rounds y to the fp4 grid step)
    nq = B - t            (= -fp4_121(|y|)*sign(y), exact subtraction)
    out = nq * (-s_t / s_b)                    (per block)

The magic add reproduces fp4_121 rounding (incl. round-half-even ties)
because the 1-2-1 grid step is 2^(e-1) clamped to >= 0.5, which equals
ulp(3*2^21 * max(2^e,1)) in fp32.

Two passes over x in HBM.  Engine split:
  ACT    issues input DMAs (HWDGE)
  SYNC   issues output + collective staging DMAs (HWDGE)
  DVE    pass-A block reduces, the int exponent trick, y and t ops
  POOL   the AllReduce, plus nq and o ops (pure-f32 tensor_tensor) for
         most tiles -- GPSIMD runs ~1.9x slower than DVE but in parallel;
         its ops are chunked so its shared-SBUF-port holds stay short.
"""

import numpy as np

FULL_SHAPE = (4, 4096, 4096)
N_CORES = 8
P = 128
TOTAL = 4 * 4096 * 4096
L = TOTAL // (N_CORES * P)   # 65536 elements per partition per core
NBLK = L // 16

EXP_MASK = 0x7F800000
MAGIC_ADD = 0x0B400000       # int-bits delta for *3*2^21
MAGIC_MIN = 0x4AC00000       # bits of 6291456.0f = 3*2^21 (= B for |y|<1)


def build_nc(L=L, F=2048, n_cores=N_CORES, n_xa=4, n_o=5,
             gp_chunk=1024, nq_dve_mod=4, gp_enable=False, fence=True):
    """nq_dve_mod: DVE keeps the nq op on every (nq_dve_mod)-th tile to
    balance the two engines; all other tiles' nq plus every tile's o op
    run on GPSIMD. gp_enable=False reverts to the all-DVE pipeline."""
    from contextlib import ExitStack

    import concourse.bass as bass
    from concourse import mybir

    f32 = mybir.dt.float32
    i32 = mybir.dt.int32
    f8 = mybir.dt.float8e4

    T = L // F
    nblk = L // 16
    fblk = F // 16
    gp_chunk = min(gp_chunk, F)
    assert L % F == 0 and F % 16 == 0 and F % gp_chunk == 0
    nch = F // gp_chunk
    cblk = gp_chunk // 16

    def gp_nq(t):
        return gp_enable and (t % nq_dve_mod != nq_dve_mod - 1)

    def gp_o(t):
        return gp_enable

    # precompute the gpsimd sem schedule (the DVE/SYNC programs need the
    # pool counts before the gpsimd block is traced)
    poolB_nq = [0] * T
    poolB_o = [0] * T
    pc = 0
    for t in range(T):
        if gp_nq(t):
            pc += nch
        poolB_nq[t] = pc
        if gp_o(t):
            pc += nch
        poolB_o[t] = pc

    nc = bass.Bass(num_devices=n_cores, debug=False)
    x_ext = nc.declare_dram_parameter("x", [P, L], f32, isOutput=False)
    out_ext = nc.declare_dram_parameter("out", [P, L], f32, isOutput=True)
    cc_in = nc.dram_tensor("cc_in", [1, 128], f32)
    cc_out = nc.dram_tensor("cc_out", [1, 128], f32, addr_space="Shared")
    cc_warm_in = nc.dram_tensor("cc_warm_in", [1, 128], f32)
    cc_warm_out = nc.dram_tensor("cc_warm_out", [1, 128], f32,
                                 addr_space="Shared")

    with ExitStack() as ctx:
        def sem(name):
            return ctx.enter_context(nc.semaphore(name))

        def sbuf(name, shape, dt=f32):
            return ctx.enter_context(nc.sbuf_tensor(name, shape, dt))

        # one sem per buffer slot: concurrent DMAs complete out of order,
        # so a shared cumulative sem cannot prove WHICH tile landed.
        s_xa = [sem(f"s_xa{i}") for i in range(n_xa)]   # in-slot DMAs  (+16)
        s_ob = [sem(f"s_ob{i}") for i in range(n_o)]    # out-slot DMAs (+16)
        s_cdma = sem("s_cdma")   # collective staging DMAs      (+16)
        s_dve = sem("s_dve")     # tagged DVE ops               (+1)
        s_pool = sem("s_pool")   # pool ops                     (+1)
        s_cc = sem("s_cc")       # collective                   (+1)
        s_warm = sem("s_warm")   # warm-up staging dma          (+16)
        assert T >= n_xa and T >= n_o and T >= 3

        xa = [sbuf(f"xa{i}", [P, F]) for i in range(n_xa)]
        yb = [sbuf(f"yb{i}", [P, F]) for i in range(2)]
        pb = [sbuf(f"pb{i}", [P, F], i32) for i in range(2)]
        tb = [sbuf(f"tb{i}", [P, F]) for i in range(2)]
        ng = ([sbuf(f"ng{i}", [P, F]) for i in range(2)]
              if gp_enable else [])
        ob = [sbuf(f"ob{i}", [P, F]) for i in range(n_o)]
        m_t = sbuf("m_t", [P, nblk])
        rm_t = sbuf("rm_t", [P, nblk])
        s1_t = sbuf("s1_t", [P, nblk])
        f8_t = sbuf("f8_t", [P, nblk], f8)
        c_t = sbuf("c_t", [P, nblk])
        nic_t = sbuf("nic_t", [P, nblk])
        mx_t = sbuf("mx_t", [P, 1])
        gall_t = sbuf("gall_t", [P, 128])
        g128_t = sbuf("g128_t", [P, 1])
        st_t = sbuf("st_t", [P, 1])
        rt_t = sbuf("rt_t", [P, 1])
        nst_t = sbuf("nst_t", [P, 1])

        # DVE tag bookkeeping (s_dve counts of key instructions)
        dveA = [0] * T
        dveB_y = [0] * T
        dveB_t = [0] * T
        dveB_nq = [0] * T
        K_mx_box = [0]
        K_nic_box = [0]

        def b3(ap):
            return ap.rearrange("p (b s) -> p b s", s=16)

        with nc.Block() as block:

            @block.vector
            def _(dve):
                cnt = 0

                def tag(ins):
                    # tag completion on s_dve; consumers emit wait_ge on the
                    # exact tag JUST BEFORE they read (this silicon's DVE
                    # does not order dependent same-engine ops by itself).
                    # Interleaving two independent tiles' ops keeps every
                    # wait already-satisfied when it executes -> no bubbles.
                    nonlocal cnt
                    ins.then_inc(s_dve)
                    cnt += 1
                    return cnt

                # ---- pass A: per-block abs max (independent ops) ----
                for t in range(T):
                    dve.wait_ge(s_xa[t % n_xa], 16 * (t // n_xa + 1))
                    dveA[t] = tag(dve.tensor_reduce(
                        out=m_t[:, t * fblk:(t + 1) * fblk],
                        in_=b3(xa[t % n_xa][:]),
                        axis=mybir.AxisListType.X,
                        op=mybir.AluOpType.max,
                        apply_absolute_value=True,
                    ))
                # local max FIRST so the collective overlaps the reciprocal
                dve.wait_ge(s_dve, dveA[T - 1])     # all m slices written
                K_mx_box[0] = tag(dve.tensor_reduce(
                    out=mx_t[:], in_=m_t[:], axis=mybir.AxisListType.X,
                    op=mybir.AluOpType.max,
                ))
                # rm = 1/m in halves, runs while the AllReduce is in flight
                h = nblk // 2
                h0 = slice(0, h)
                h1 = slice(h, nblk)
                k_rm = [0, 0]
                k_rm[0] = tag(dve.reciprocal(rm_t[:, h0], m_t[:, h0]))
                k_rm[1] = tag(dve.reciprocal(rm_t[:, h1], m_t[:, h1]))

                # ---- scalars ----
                dve.wait_ge(s_cdma, 32)         # gall loaded (bcast DMA)
                k = tag(dve.tensor_reduce(
                    out=g128_t[:], in_=gall_t[:], axis=mybir.AxisListType.X,
                    op=mybir.AluOpType.max))
                dve.wait_ge(s_dve, k)
                k_st = tag(dve.tensor_scalar(
                    st_t[:], g128_t[:], 1.0 / 2688.0, None,
                    op0=mybir.AluOpType.mult))
                dve.wait_ge(s_dve, k_st)

                # ---- per-block scales, halves interleaved ----
                k_inv = [0, 0]
                k_inv[0] = tag(dve.tensor_scalar(
                    s1_t[:, h0], rm_t[:, h0], st_t[:], 6.0,
                    op0=mybir.AluOpType.mult, op1=mybir.AluOpType.mult))
                k_rt = tag(dve.reciprocal(rt_t[:], st_t[:]))
                k_inv[1] = tag(dve.tensor_scalar(
                    s1_t[:, h1], rm_t[:, h1], st_t[:], 6.0,
                    op0=mybir.AluOpType.mult, op1=mybir.AluOpType.mult))
                k_nst = tag(dve.tensor_scalar(
                    nst_t[:], st_t[:], -1.0, None, op0=mybir.AluOpType.mult))
                k_f8 = [0, 0]
                k_up = [0, 0]
                k_eq = [0, 0]
                k_sb = [0, 0]
                k_c = [0, 0]
                k_rs = [0, 0]
                k_nic = [0, 0]
                for j, hs in ((0, h0), (1, h1)):
                    f8h = slice(hs.start // 16, (hs.start + h) // 16)
                    dve.wait_ge(s_dve, k_inv[j])
                    k_f8[j] = tag(dve.tensor_copy(f8_t[:, hs], s1_t[:, hs]))
                for j, hs in ((0, h0), (1, h1)):
                    dve.wait_ge(s_dve, k_f8[j])
                    k_up[j] = tag(dve.tensor_copy(m_t[:, hs], f8_t[:, hs]))
                for j, hs in ((0, h0), (1, h1)):
                    dve.wait_ge(s_dve, k_up[j])
                    k_eq[j] = tag(dve.tensor_scalar(
                        s1_t[:, hs], m_t[:, hs], 0.0, None,
                        op0=mybir.AluOpType.is_equal))
                for j, hs in ((0, h0), (1, h1)):
                    dve.wait_ge(s_dve, k_eq[j])
                    k_sb[j] = tag(dve.tensor_tensor(
                        rm_t[:, hs], m_t[:, hs], s1_t[:, hs],
                        op=mybir.AluOpType.add))
                for j, hs in ((0, h0), (1, h1)):
                    dve.wait_ge(s_dve, k_sb[j])
                    k_c[j] = tag(dve.tensor_scalar(
                        c_t[:, hs], rm_t[:, hs], rt_t[:], None,
                        op0=mybir.AluOpType.mult))
                for j, hs in ((0, h0), (1, h1)):
                    k_rs[j] = tag(dve.reciprocal(m_t[:, hs], rm_t[:, hs]))
                for j, hs in ((0, h0), (1, h1)):
                    dve.wait_ge(s_dve, k_rs[j])
                    k_nic[j] = tag(dve.tensor_scalar(
                        nic_t[:, hs], m_t[:, hs], nst_t[:], None,
                        op0=mybir.AluOpType.mult))
                K_nic_box[0] = k_nic[1]

                # ---- pass B: pairs of tiles, ops interleaved ----
                tag_y = [0] * T
                tag_p = [0] * T
                tag_b = [0] * T
                tag_t = [0] * T
                tag_nq = [0] * T
                for tp in range(0, T, 2):
                    pair = (tp, tp + 1)
                    for t in pair:
                        g = T + t
                        dve.wait_ge(s_xa[g % n_xa], 16 * (g // n_xa + 1))
                        if t >= n_o:
                            dve.wait_ge(s_ob[t % n_o],
                                        16 * ((t - n_o) // n_o + 1))
                    for t in pair:
                        if t >= 2:
                            # tile t-2 fully retired: frees yb/pb/tb[t%2]
                            dve.wait_ge(s_dve, dveB_nq[t - 2])
                        bsl = slice(t * fblk, (t + 1) * fblk)
                        tag_y[t] = tag(dve.tensor_tensor(
                            b3(yb[t % 2][:]), b3(xa[(T + t) % n_xa][:]),
                            c_t[:, bsl].unsqueeze(-1).broadcast_to(
                                [P, fblk, 16]),
                            op=mybir.AluOpType.mult))
                        dveB_y[t] = tag_y[t]
                    for t in pair:
                        dve.wait_ge(s_dve, tag_y[t])
                        tag_p[t] = tag(dve.tensor_scalar(
                            pb[t % 2][:], yb[t % 2][:].bitcast(i32),
                            EXP_MASK, None,
                            op0=mybir.AluOpType.bitwise_and))
                    for t in pair:
                        dve.wait_ge(s_dve, tag_p[t])
                        tag_b[t] = tag(dve.tensor_scalar(
                            pb[t % 2][:], pb[t % 2][:], MAGIC_ADD, MAGIC_MIN,
                            op0=mybir.AluOpType.add,
                            op1=mybir.AluOpType.max))
                    for t in pair:
                        dve.wait_ge(s_dve, tag_b[t])
                        tag_t[t] = tag(dve.tensor_tensor(
                            tb[t % 2][:], yb[t % 2][:],
                            pb[t % 2][:].bitcast(f32),
                            op=mybir.AluOpType.add))
                    for t in pair:
                        dve.wait_ge(s_dve, tag_t[t])
                        tag_nq[t] = tag(dve.tensor_tensor(
                            yb[t % 2][:], pb[t % 2][:].bitcast(f32),
                            tb[t % 2][:], op=mybir.AluOpType.subtract))
                    for t in pair:
                        bsl = slice(t * fblk, (t + 1) * fblk)
                        dve.wait_ge(s_dve, tag_nq[t])
                        if tp == 0:
                            dve.wait_ge(s_dve, K_nic_box[0])
                        dveB_nq[t] = tag(dve.tensor_tensor(
                            b3(ob[t % n_o][:]), b3(yb[t % 2][:]),
                            nic_t[:, bsl].unsqueeze(-1).broadcast_to(
                                [P, fblk, 16]),
                            op=mybir.AluOpType.mult))

            @block.gpsimd
            def _(pool):
                # warm-up collective: absorbs the ~20us first-use firmware
                # wake while pass A streams; the real AllReduce then starts
                # on a hot path.
                pool.memset(gall_t[0:1, :], 0.0).then_inc(s_pool)
                pool.wait_ge(s_pool, 1)
                pool.dma_start(out=cc_warm_in[:, :],
                               in_=gall_t[0:1, :]).then_inc(s_warm, 16)
                pool.wait_ge(s_warm, 16)
                pool.collective_compute(
                    "AllReduce",
                    mybir.AluOpType.max,
                    replica_groups=[list(range(n_cores))],
                    ins=[cc_warm_in.ap().opt()],
                    outs=[cc_warm_out.ap().opt()],
                ).then_inc(s_cc)
                pool.wait_ge(s_cdma, 16)        # cc_in staged
                pool.collective_compute(
                    "AllReduce",
                    mybir.AluOpType.max,
                    replica_groups=[list(range(n_cores))],
                    ins=[cc_in.ap().opt()],
                    outs=[cc_out.ap().opt()],
                ).then_inc(s_cc)
                if not gp_enable:
                    return
                pcnt = 0

                def pinc(ins):
                    nonlocal pcnt
                    ins.then_inc(s_pool)
                    pcnt += 1
                    pool.wait_ge(s_pool, pcnt)
                    return pcnt

                pool.wait_ge(s_dve, K_nic_box[0])   # nic ready
                for t in range(T):
                    y, p, tbuf, nq, o = (yb[t % 2], pb[t % 2], tb[t % 2],
                                         ng[t % 2], ob[t % n_o])
                    if gp_nq(t):
                        pool.wait_ge(s_dve, dveB_t[t])
                        for k in range(nch):
                            csl = slice(k * gp_chunk, (k + 1) * gp_chunk)
                            pinc(pool.tensor_tensor(
                                nq[:, csl], p[:, csl].bitcast(f32),
                                tbuf[:, csl], op=mybir.AluOpType.subtract))
                        src = nq
                        assert pcnt == poolB_nq[t]
                    else:
                        pool.wait_ge(s_dve, dveB_nq[t])
                        src = y
                    if t >= n_o:
                        pool.wait_ge(s_ob[t % n_o],
                                     16 * ((t - n_o) // n_o + 1))
                    for k in range(nch):
                        ca = slice(k * cblk + t * fblk,
                                   (k + 1) * cblk + t * fblk)
                        ks = slice(k * cblk, (k + 1) * cblk)
                        pinc(pool.tensor_tensor(
                            b3(o[:])[:, ks],
                            b3(src[:])[:, ks],
                            nic_t[:, ca].unsqueeze(-1).broadcast_to(
                                [P, cblk, 16]),
                            op=mybir.AluOpType.mult))
                    assert pcnt == poolB_o[t]

            @block.scalar
            def _(act):
                # pass A input DMAs
                for t in range(T):
                    if t >= n_xa:
                        act.wait_ge(s_dve, dveA[t - n_xa])
                    act.dma_start(
                        out=xa[t % n_xa][:, :],
                        in_=x_ext[:, t * F:(t + 1) * F],
                    ).then_inc(s_xa[t % n_xa], 16)
                # pass B input DMAs (re-read)
                for t in range(T):
                    if t >= n_xa:
                        act.wait_ge(s_dve, dveB_y[t - n_xa])
                    else:
                        act.wait_ge(s_dve, dveA[T - n_xa + t])
                    act.dma_start(
                        out=xa[(T + t) % n_xa][:, :],
                        in_=x_ext[:, t * F:(t + 1) * F],
                    ).then_inc(s_xa[(T + t) % n_xa], 16)

            @block.sync
            def _(sync):
                sync.wait_ge(s_dve, K_mx_box[0])
                sync.dma_start(out=cc_in[:, :], in_=mx_t[:, :]).then_inc(
                    s_cdma, 16)
                sync.wait_ge(s_cc, 2)
                sync.dma_start(
                    out=gall_t[:, :],
                    in_=cc_out.ap().broadcast_to([P, 128]),
                ).then_inc(s_cdma, 16)
                for t in range(T):
                    if gp_o(t):
                        sync.wait_ge(s_pool, poolB_o[t])
                    else:
                        sync.wait_ge(s_dve, dveB_nq[t])
                    sync.dma_start(
                        out=out_ext[:, t * F:(t + 1) * F],
                        in_=ob[t % n_o][:, :],
                    ).then_inc(s_ob[t % n_o], 16)
                for i in range(n_o):
                    uses = len([t for t in range(T) if t % n_o == i])
                    sync.wait_ge(s_ob[i], 16 * uses)

    return nc


_CACHE = {}


def _get_nc():
    if "nc" not in _CACHE:
        _CACHE["nc"] = build_nc()
    return _CACHE["nc"]


def kernel(x: np.ndarray) -> np.ndarray:
    from concourse.bass_utils import run_bass_kernel_spmd

    x = np.asarray(x, dtype=np.float32)
    assert x.shape == FULL_SHAPE
    shards = x.reshape(N_CORES, P, L)
    in_maps = [{"x": np.ascontiguousarray(shards[i])} for i in range(N_CORES)]
    nc = _get_nc()
    res = run_bass_kernel_spmd(nc, in_maps, core_ids=list(range(N_CORES)))
    out = np.stack([r["out"] for r in res.results], axis=0)
    return out.reshape(FULL_SHAPE)
